# revision 30
# baseline (speedup 1.0000x reference)
"""Trainium2 Bass kernel for the batch ConsistencyLoss (masked pairwise KL).

Math (reference):
    emb = x / ||x||;  sim = emb @ emb.T;  mask = (sim > 0.8) & ~eye
    L = log_softmax(routing);  P = exp(L);  ne[j] = sum_k P[j,k] L[j,k]
    kl[i,j] = ne[j] - L_i . P_j
    loss = sum(mask * kl) / count(mask)

Key ideas vs a row-strip baseline:
  * Upper-triangle only: mask is symmetric and
        kl[i,j] + kl[j,i] = ne_i + ne_j - L_i.P_j - P_i.L_j
    is symmetric in (i,j), so each unordered block pair {a,b} of the
    16x512-row block grid is computed once, halving the dominant
    sim matmul.  Per-core assignment (8 cores x 17 block tasks) is made
    SPMD-uniform by gathering per-core block lists on the host into one
    input; every core runs the identical program over local positions.
  * Both matmul operands are scaled by C/||x|| during the f32->fp8
    conversion, so sim PSUM holds C^2*cos directly and the mask is a
    single constant-threshold compare (no norm broadcasts).
  * fp8(e4m3) DoubleRow matmuls: 2 k-subtiles per instruction.
  * Masked-KL factorization per task via one [34-wide] matmul:
        U = [L|P|ne|1]_rows^T @ mask  ->  S += sum_j F_cols (x) U^T
    with F = [-P|-L|1|ne]; pair count rides along as U's last row.
"""

import numpy as np

import concourse.bacc as bacc
import concourse.tile as tile
from concourse import mybir
from concourse.bass_utils import run_bass_kernel_spmd
from concourse.masks import make_identity

B, E, H = 8192, 16, 1024
NCORES = 8
NB = 16          # 512-row blocks of the batch
BS = 512
KT = H // 128    # 8 contraction chunks of 128
GN = 11          # gathered 512-row groups per core
GC = GN * 4      # 44 chunks of 128 rows
GROWS = GN * BS  # 5632 gathered rows
CS = 16.0        # fp8 scale: rows scaled by CS/||x||
THR = 0.8 * CS * CS
WEIGHT = 1.0

F32 = mybir.dt.float32
BF16 = mybir.dt.bfloat16
FP8 = mybir.dt.float8e4
I32 = mybir.dt.int32
AX = mybir.AxisListType.X
AXY = mybir.AxisListType.XY
OP = mybir.AluOpType
AF = mybir.ActivationFunctionType
DR = mybir.MatmulPerfMode.DoubleRow

# 17 uniform tasks over local group positions (rows_pos, cols_pos, is_diag),
# ordered so early-prepped groups unblock compute first.
TASKS = [(0, 0, True), (0, 1, False), (1, 1, True)]
for _c in range(2, 8):
    TASKS += [(0, _c, False), (1, _c, False)]
TASKS += [(1, 8, False), (9, 10, False)]
assert len(TASKS) == 17
GB_BASE = {0: 0, 1: 4, 9: 8}  # rows position -> Gb chunk base


def core_blocks(d: int) -> list:
    blocks = [(2 * d + o) % NB for o in range(9)]
    if d < 4:
        a, b = 2 * d, (2 * d + 8) % NB
    else:
        a, b = (2 * d + 1) % NB, (2 * d + 9) % NB
    return blocks + [a, b]


def _check_coverage():
    seen = {}
    for d in range(NCORES):
        bl = core_blocks(d)
        for (pr, pc, diag) in TASKS:
            assert diag == (bl[pr] == bl[pc])
            key = (min(bl[pr], bl[pc]), max(bl[pr], bl[pc]))
            seen[key] = seen.get(key, 0) + 1
    assert sorted(seen) == [(a, b) for a in range(NB) for b in range(a, NB)]
    assert all(v == 1 for v in seen.values())


_check_coverage()


def _kernel(tc, embg, rpg, out_dram, reps=1, loop_iters=None, upto="Z"):
    nc = tc.nc
    with tc.tile_pool(name="persist", bufs=1) as persist:
        embt = persist.tile([128, KT, GROWS], FP8)   # scaled emb^T chunks
        F_all = persist.tile([128, GC, 34], F32)     # [-P|-L|1|ne] per row
        Gb = persist.tile([128, 12, 34], BF16)       # [L|P|ne|1] row groups
        Ut_all = persist.tile([128, 68, 34], F32)    # U^T per task j-chunk
        Q = persist.tile([128, 896], BF16)           # strict-upper patterns
        io = persist.tile([128, 896], I32)
        identb = persist.tile([128, 128], BF16)
        identf = persist.tile([128, 128], F32)
        ones = persist.tile([128, 1], F32)
        ss = persist.tile([128, GC], F32)            # sum of squares / row
        nrm = persist.tile([128, GC], F32)
        rn = persist.tile([128, GC], F32)            # CS / ||x||

        make_identity(nc, identb)
        make_identity(nc, identf)
        nc.vector.memset(ones, 1.0)
        nc.gpsimd.iota(io, pattern=[[1, 896]], base=-384,
                       channel_multiplier=-1)
        nc.vector.tensor_scalar(Q, io, 0, None, op0=OP.is_gt)
        nc.vector.memset(F_all[:, :, 32:33], 1.0)
        nc.vector.memset(Gb[:, :, 33:34], 1.0)

        args = (tc, nc, embg, rpg, out_dram, embt, F_all, Gb, Ut_all, Q,
                identb, identf, ones, ss, nrm, rn)
        if loop_iters is not None:
            with tc.For_i(0, loop_iters, 1):
                _phases(*args, "", upto)
            return
        for rep in range(reps):
            _phases(*args, f"r{rep}_" if reps > 1 else "", upto)


def _phases(tc, nc, embg, rpg, out_dram, embt, F_all, Gb, Ut_all, Q,
            identb, identf, ones, ss, nrm, rn, r, upto="Z"):
    # ---- Phase A: softmax stats for all 44 gathered chunks ----
    with tc.tile_pool(name=f"{r}smx", bufs=1) as smx:
        rp_sb = smx.tile([128, GC, E], F32, tag="rp_sb")
        nc.sync.dma_start(
            out=rp_sb, in_=rpg.rearrange("(c p) e -> p c e", p=128))
        e_all = smx.tile([128, GC, E], F32, tag="e_all")
        s_all = smx.tile([128, GC], F32, tag="s_all")
        logs_all = smx.tile([128, GC], F32, tag="logs_all")
        rs_all = smx.tile([128, GC], F32, tag="rs_all")
        for c in range(GC):
            nc.scalar.activation(out=e_all[:, c, :], in_=rp_sb[:, c, :],
                                 func=AF.Exp, bias=0.0, scale=1.0,
                                 accum_out=s_all[:, c:c + 1])
        nc.scalar.activation(out=logs_all, in_=s_all, func=AF.Ln)
        nc.vector.reciprocal(out=rs_all, in_=s_all)
        for c in range(GC):
            # F[:, c, 0:16] = -P;  F[:, c, 16:32] = -L
            nc.vector.tensor_scalar(F_all[:, c, 0:16], e_all[:, c, :],
                                    rs_all[:, c:c + 1], -1.0,
                                    op0=OP.mult, op1=OP.mult)
            nc.vector.tensor_scalar(F_all[:, c, 16:32], rp_sb[:, c, :],
                                    logs_all[:, c:c + 1], -1.0,
                                    op0=OP.subtract, op1=OP.mult)
        # ne = sum_k P*L = sum (-P)(-L), batched over all chunks
        scr3 = smx.tile([128, GC, E], F32, tag="scr3")
        nc.vector.tensor_tensor(out=scr3, in0=F_all[:, :, 0:16],
                                in1=F_all[:, :, 16:32], op=OP.mult)
        ne_t = smx.tile([128, GC, 1], F32, tag="ne_t")
        nc.vector.reduce_sum(out=ne_t, in_=scr3, axis=AX)
        nc.vector.tensor_copy(out=F_all[:, :, 33:34], in_=ne_t)
        # Gb rows: positions {0,1,9} -> chunks {0..7, 36..39}
        for rr in range(12):
            c = rr if rr < 8 else 28 + rr
            nc.vector.tensor_scalar(Gb[:, rr, 0:16], rp_sb[:, c, :],
                                    logs_all[:, c:c + 1], None,
                                    op0=OP.subtract)
            nc.vector.tensor_scalar(Gb[:, rr, 16:32], e_all[:, c, :],
                                    rs_all[:, c:c + 1], None, op0=OP.mult)
        nc.vector.tensor_copy(out=Gb[:, 0:8, 32:33], in_=F_all[:, 0:8, 33:34])
        nc.vector.tensor_copy(out=Gb[:, 8:12, 32:33],
                              in_=F_all[:, 36:40, 33:34])

    if upto == "A":
        with tc.tile_pool(name=f"{r}dbg", bufs=1) as dbg:
            a2 = dbg.tile([128, 2], F32)
            nc.vector.reduce_sum(out=a2[:, 0:1], in_=F_all, axis=AXY)
            nc.vector.reduce_sum(out=a2[:, 1:2], in_=Gb, axis=AXY)
            nc.sync.dma_start(out=out_dram, in_=a2[0:1, :])
        return

    # ---- Phases B+C+D interleaved: prep group g, then emit every task
    # whose operand groups are now ready, so PE/DVE/ACT/DMA pipeline. ----
    with tc.tile_pool(name=f"{r}prep", bufs=1) as prep, \
         tc.tile_pool(name=f"{r}task", bufs=1) as taskp, \
         tc.tile_pool(name=f"{r}fin", bufs=1) as fin, \
         tc.tile_pool(name=f"{r}trps", bufs=2, space="PSUM") as trps, \
         tc.tile_pool(name=f"{r}sps", bufs=2, space="PSUM") as sps, \
         tc.tile_pool(name=f"{r}ups", bufs=2, space="PSUM") as ups, \
         tc.tile_pool(name=f"{r}utps", bufs=1, space="PSUM") as utps:

        accs = fin.tile([1, 18], F32)

        def prep_group(g):
            xs = []
            for cc in range(4):
                c = 4 * g + cc
                x = prep.tile([128, H], F32, tag="x", bufs=6)
                xs.append(x)
                nc.sync.dma_start(out=x, in_=embg[c * 128:(c + 1) * 128, :])
                sqs = prep.tile([128, H], F32, tag="sqs", bufs=2)
                nc.scalar.activation(out=sqs, in_=x, func=AF.Square,
                                     bias=0.0, scale=1.0,
                                     accum_out=ss[:, c:c + 1])
            g4 = slice(4 * g, 4 * g + 4)
            nc.scalar.activation(out=nrm[:, g4], in_=ss[:, g4], func=AF.Sqrt,
                                 bias=0.0, scale=1.0 / (CS * CS))
            nc.vector.reciprocal(out=rn[:, g4], in_=nrm[:, g4])
            for cc in range(4):
                c = 4 * g + cc
                xb = prep.tile([128, H], BF16, tag="xb", bufs=2)
                nc.vector.tensor_scalar(xb, xs[cc], rn[:, c:c + 1], None,
                                        op0=OP.mult)
                tp = trps.tile([128, H], BF16, tag="tp")
                for kt in range(KT):
                    nc.tensor.transpose(tp[:, kt * 128:(kt + 1) * 128],
                                        xb[:, kt * 128:(kt + 1) * 128],
                                        identb)
                nc.vector.tensor_copy(
                    out=embt[:, :, c * 128:(c + 1) * 128],
                    in_=tp.rearrange("p (k q) -> p k q", k=KT))

        def do_task(t):
            pr, pc, diag = TASKS[t]
            u = ups.tile([34, 512], F32, tag="u")
            for m in range(4):
                rs0 = (pr * 4 + m) * 128
                cs0 = pc * 512
                sim = sps.tile([128, 512], F32, tag="sim")
                for t2 in range(4):
                    nc.tensor.matmul(
                        out=sim,
                        lhsT=embt[:, 2 * t2:2 * t2 + 2, rs0:rs0 + 128],
                        rhs=embt[:, 2 * t2:2 * t2 + 2, cs0:cs0 + 512],
                        start=(t2 == 0), stop=(t2 == 3), perf_mode=DR)
                msk = taskp.tile([128, 512], BF16, tag="msk", bufs=3)
                if diag:
                    off = (3 - m) * 128
                    nc.vector.scalar_tensor_tensor(
                        out=msk, in0=sim, scalar=THR,
                        in1=Q[:, off:off + 512], op0=OP.is_gt, op1=OP.mult)
                else:
                    nc.vector.tensor_scalar(msk, sim, THR, None,
                                            op0=OP.is_gt)
                nc.tensor.matmul(out=u, lhsT=Gb[:, GB_BASE[pr] + m, :],
                                 rhs=msk, start=(m == 0), stop=(m == 3))
            ust = taskp.tile([34, 512], F32, tag="ust", bufs=2)
            nc.scalar.copy(out=ust, in_=u)
            for q in range(4):
                utp = utps.tile([128, 34], F32, tag="utp")
                nc.tensor.matmul(out=utp, lhsT=ust[:, q * 128:(q + 1) * 128],
                                 rhs=identf[:34, :34], start=True, stop=True)
                nc.scalar.copy(out=Ut_all[:, 4 * t + q, :], in_=utp)
            # final contraction for this task on gpsimd (vector would
            # head-of-line block evacs; tensor_tensor_reduce wedges the
            # device in this flow)
            scrT = fin.tile([128, 4, 34], F32, tag="scrT", bufs=2)
            nc.gpsimd.tensor_tensor(
                out=scrT, in0=F_all[:, pc * 4:pc * 4 + 4, :],
                in1=Ut_all[:, 4 * t:4 * t + 4, :], op=OP.mult)
            nc.gpsimd.tensor_reduce(out=accs[:, t:t + 1], in_=scrT,
                                    axis=mybir.AxisListType.XYZWC,
                                    op=OP.add)

        # tasks become ready once their max group position is prepped;
        # keep prep AHEAD groups in front of task consumption so the
        # in-order engine queues never head-of-line block on task chains.
        AHEAD = 2  # GN = fully phase-separated; small = interleaved
        ready_at = {}
        for t, (pr, pc, _) in enumerate(TASKS):
            ready_at.setdefault(max(pr, pc), []).append(t)
        for g in range(GN + AHEAD):
            if g < GN:
                prep_group(g)
            if g - AHEAD >= 0:
                for t in ready_at.get(g - AHEAD, []):
                    do_task(t)

        nc.gpsimd.tensor_reduce(out=accs[:, 17:18], in_=Ut_all[:, :, 33:34],
                                axis=mybir.AxisListType.XYZWC, op=OP.add)
        out_sb = fin.tile([1, 2], F32)
        nc.vector.reduce_sum(out=out_sb[0:1, 0:1], in_=accs[0:1, 0:17],
                             axis=AX)
        nc.vector.tensor_copy(out=out_sb[0:1, 1:2], in_=accs[:, 17:18])
        nc.sync.dma_start(out=out_dram, in_=out_sb)


def build_bass(reps=1, loop_iters=None, upto="Z"):
    nc = bacc.Bacc("TRN2", target_bir_lowering=False, debug=False)
    embg = nc.dram_tensor("embg", [GROWS, H], F32, kind="ExternalInput").ap()
    rpg = nc.dram_tensor("rpg", [GROWS, E], F32, kind="ExternalInput").ap()
    out = nc.dram_tensor("out", [1, 2], F32, kind="ExternalOutput").ap()
    with tile.TileContext(nc) as tc:
        _kernel(tc, embg, rpg, out, reps=reps, loop_iters=loop_iters,
                upto=upto)
    nc.compile()
    return nc


_NC_CACHE = None


def make_in_map(rp: np.ndarray, emb: np.ndarray, d: int) -> dict:
    ev = emb.reshape(NB, BS, H)
    rv = rp.reshape(NB, BS, E)
    bl = core_blocks(d)
    return {
        "embg": np.concatenate([ev[g] for g in bl], 0),
        "rpg": np.concatenate([rv[g] for g in bl], 0),
    }


def kernel(routing_probs: np.ndarray, input_embeddings: np.ndarray,
           **_unused) -> np.ndarray:
    global _NC_CACHE
    if _NC_CACHE is None:
        _NC_CACHE = build_bass()
    nc = _NC_CACHE
    rp = np.ascontiguousarray(routing_probs, dtype=np.float32)
    emb = np.ascontiguousarray(input_embeddings, dtype=np.float32)
    in_maps = [make_in_map(rp, emb, d) for d in range(NCORES)]
    res = run_bass_kernel_spmd(nc, in_maps, core_ids=list(range(NCORES)))
    vals = np.array([r["out"].reshape(2) for r in res.results],
                    dtype=np.float64)
    s_tot = vals[:, 0].sum()
    cnt = vals[:, 1].sum()  # unordered masked pairs
    if cnt > 0:
        loss = np.float32(s_tot) / np.float32(2.0 * cnt)
    else:
        loss = 0.0
    return np.array(WEIGHT * loss, dtype=np.float32)
